# revision 3
# baseline (speedup 1.0000x reference)
"""Trainium2 Bass kernel for ContinuousREWAEncoder:
    out = FWHT(x @ W^T)/sqrt(32) + 0.01*normal(key=42)

Math folding: FWHT is linear => out = x @ (H @ W / sqrt(32))^T + noise.
The noise uses a fixed PRNG key, so it is a deterministic constant computed
on host (with the same jax op/backend as the reference) and added in the
host epilogue (with the layout unpermute), keeping it off the HBM stream.

Sharding: pure data parallel over tokens (B*N = 32768 -> 4096/core on 8
cores). W_eff is replicated.

The kernel is HBM-bound, so x streams as fp8e3 (e3m4: 4 mantissa bits) —
half the bytes of fp16 — while W stays fp16 (mixed-dtype matmul). Measured
absmax rel err vs the fp32 reference ~1.1e-2 (gate 2e-2). Output moves as
fp16.

Device schedule per core (TOK=4096 = 4 supersteps x 4 col groups x 256):
  - the x stream is split across BOTH HWDGE rings (sync + scalar) so the
    16 SDMA engines never starve on descriptor supply: ring A (sync)
    carries each superstep's chunks 0-3 (4 x 512 KB) plus the final
    128 KB chunk-7 piece; ring B (scalar) carries w, chunks 4-7 of
    supersteps 0-2 and chunks 4-6 of superstep 3.
  - per superstep: 8 k-chunks x 4 col groups of matmuls (N=256), the 4
    groups run concurrently in the PE column groups (tile_position),
    accumulating into that superstep's own PSUM bank; then a [128,256]
    DVE cast and a 64 KB store. 4 banks -> casts/stores of earlier
    supersteps pipeline under the continuing x stream, and after the
    last x byte only 4 N=256 matmuls + one cast + one 64 KB store remain.
  - out stores alternate rings (sync/scalar) behind the x issues.
"""

import math

import numpy as np
import ml_dtypes

import concourse.tile as tile
from concourse import bacc, mybir
from concourse.bass_utils import run_bass_kernel_spmd

B, N, D, M = 4, 8192, 1024, 32
NOISE_STD = 0.01
N_CORES = 8
TOK_TOTAL = B * N              # 32768
TOK = TOK_TOTAL // N_CORES     # 4096 tokens per core
BLK = 256                      # tokens per PSUM column-group
NGRP = 4                       # col groups per superstep (PE col tiling)
SS = TOK // (BLK * NGRP)       # 4 supersteps
KC = D // 128                  # 8 contraction chunks
KH = KC // 2                   # 4 chunks per half-DMA

X_DT = mybir.dt.float8e3       # e3m4: 1 byte, 4 mantissa bits
X_NP = ml_dtypes.float8_e3m4
W_DT = mybir.dt.float16
F16 = mybir.dt.float16
F32 = mybir.dt.float32

CHUNK_B = NGRP * BLK           # 1024 elems per (chunk, partition)


def _build_bass():
    nc = bacc.Bacc("TRN2", target_bir_lowering=False)

    # x pre-tiled on host: per slab [128, (c, g, t)] so each DMA moves one
    # fully-contiguous 4096 B (or smaller, for the tail pieces) run per
    # partition.
    xA = nc.dram_tensor("xA", [SS, 128, KH * CHUNK_B], X_DT, kind="ExternalInput")
    xA7 = nc.dram_tensor("xA7", [128, CHUNK_B], X_DT, kind="ExternalInput")
    xB = nc.dram_tensor("xB", [SS - 1, 128, KH * CHUNK_B], X_DT, kind="ExternalInput")
    xB3 = nc.dram_tensor("xB3", [128, 3 * CHUNK_B], X_DT, kind="ExternalInput")
    wT = nc.dram_tensor("wT", [128, KC * M], W_DT, kind="ExternalInput")
    # out rows 128*s + 32*j + m  =  (superstep s, col group j, channel m)
    outT = nc.dram_tensor("outT", [SS * NGRP * M, BLK], F16, kind="ExternalOutput")

    with tile.TileContext(nc) as tc:
        with (
            tc.tile_pool(name="w", bufs=1) as wpool,
            tc.tile_pool(name="x", bufs=1) as xpool,
            tc.tile_pool(name="out", bufs=1) as opool,
            tc.tile_pool(name="warm", bufs=1, space="PSUM") as warmpool,
            tc.tile_pool(name="psum", bufs=1, space="PSUM") as ppool,
        ):
            # w rides ring B (scalar) ahead of its x slabs.
            w_tile = wpool.tile([128, KC, M], W_DT)
            nc.scalar.dma_start(w_tile[:], wT.rearrange("p (c m) -> p c m", c=KC))

            # ring A (sync): chunks 0-3 of each superstep, then s3 chunk 7.
            a_tiles = []
            for s in range(SS):
                t = xpool.tile([128, KH, NGRP, BLK], X_DT, tag="xa", bufs=SS)
                nc.sync.dma_start(
                    t[:],
                    xA[s].rearrange("p (c g t) -> p c g t", c=KH, g=NGRP),
                )
                a_tiles.append(t)
            c7_tile = xpool.tile([128, NGRP, BLK], X_DT, tag="xc7")
            nc.sync.dma_start(
                c7_tile[:], xA7.rearrange("p (g t) -> p g t", g=NGRP)
            )

            # ring B (scalar): chunks 4-7 of supersteps 0-2, chunks 4-6 of s3.
            b_tiles = []
            for s in range(SS - 1):
                t = xpool.tile([128, KH, NGRP, BLK], X_DT, tag="xb", bufs=SS - 1)
                nc.scalar.dma_start(
                    t[:],
                    xB[s].rearrange("p (c g t) -> p c g t", c=KH, g=NGRP),
                )
                b_tiles.append(t)
            b3_tile = xpool.tile([128, 3, NGRP, BLK], X_DT, tag="xb3")
            nc.scalar.dma_start(
                b3_tile[:], xB3.rearrange("p (c g t) -> p c g t", c=3, g=NGRP)
            )

            # Warmup matmul absorbs the w-DMA wait into PE program order so
            # every real matmul needs only its x-DMA wait.
            warm = warmpool.tile([M, M], F32)
            nc.tensor.matmul(warm[:], w_tile[:, 0, :], w_tile[:, 0, :])

            for s in range(SS):
                ptile = ppool.tile([128, BLK], F32, tag=f"ps{s}")
                o_tile = opool.tile([128, BLK], F16, tag=f"o{s}")
                row = s * NGRP * M
                for c in range(KC):
                    for j in range(NGRP):
                        if c < KH:
                            rhs = a_tiles[s][:, c, j, :]
                        elif s == SS - 1:
                            rhs = (
                                c7_tile[:, j, :]
                                if c == KC - 1
                                else b3_tile[:, c - KH, j, :]
                            )
                        else:
                            rhs = b_tiles[s][:, c - KH, j, :]
                        nc.tensor.matmul(
                            ptile[32 * j : 32 * (j + 1), :],
                            w_tile[:, c, :],
                            rhs,
                            start=(c == 0),
                            stop=(c == KC - 1),
                            tile_position=(0, 32 * j),
                        )

                nc.vector.tensor_copy(o_tile[:], ptile[:])
                eng = nc.sync if s % 2 == 0 else nc.scalar
                eng.dma_start(outT[row : row + NGRP * M], o_tile[:])

    nc.compile()
    return nc


_NC_CACHE = None


def _get_nc():
    global _NC_CACHE
    if _NC_CACHE is None:
        _NC_CACHE = _build_bass()
    return _NC_CACHE


def _hadamard32() -> np.ndarray:
    h = np.array([[1.0]], dtype=np.float64)
    while h.shape[0] < M:
        h = np.block([[h, h], [h, -h]])
    return h


_NOISE_CACHE = None


def _noise() -> np.ndarray:
    # Mirror reference.py exactly (same op on the default jax backend): the
    # bits differ between backends, so the noise must be produced the same
    # way the grading reference produces it.
    global _NOISE_CACHE
    if _NOISE_CACHE is None:
        import jax

        nz = NOISE_STD * jax.random.normal(
            jax.random.key(42), (B, N, M), dtype=np.float32
        )
        _NOISE_CACHE = np.asarray(nz)
    return _NOISE_CACHE


def kernel(x: np.ndarray, W: np.ndarray, _profile_sink=None) -> np.ndarray:
    x = np.ascontiguousarray(np.asarray(x, dtype=np.float32))
    W = np.asarray(W, dtype=np.float32)

    # Fold normalized FWHT into the projection: out = x @ w_lhsT + noise
    w_eff = (_hadamard32() @ W.astype(np.float64)) / math.sqrt(M)
    w_lhsT = w_eff.T.astype(np.float16)  # [D, M]
    # pack to device SBUF layout [partition, kchunk, M]
    w_dev = np.ascontiguousarray(
        w_lhsT.reshape(KC, 128, M).transpose(1, 0, 2)
    ).reshape(128, KC * M)

    X8 = x.reshape(TOK_TOTAL, D).astype(X_NP)

    in_maps = []
    for i in range(N_CORES):
        sl = slice(i * TOK, (i + 1) * TOK)
        # [tok, d] -> [s, c, p, g, t]; slab (s, crange) = [128, (c g t)]
        xt = (
            X8[sl]
            .reshape(SS, NGRP, BLK, KC, 128)      # [s, g, t, c, p]
            .transpose(0, 3, 4, 1, 2)             # [s, c, p, g, t]
        )
        xa = np.ascontiguousarray(
            xt[:, :KH].transpose(0, 2, 1, 3, 4)   # [s, p, c, g, t]
        ).reshape(SS, 128, KH * CHUNK_B)
        xb_full = np.ascontiguousarray(
            xt[:, KH:].transpose(0, 2, 1, 3, 4)
        ).reshape(SS, 128, KH * CHUNK_B)
        in_maps.append(
            {
                "xA": xa,
                "xA7": np.ascontiguousarray(
                    xb_full[SS - 1, :, 3 * CHUNK_B :]
                ),
                "xB": np.ascontiguousarray(xb_full[: SS - 1]),
                "xB3": np.ascontiguousarray(
                    xb_full[SS - 1, :, : 3 * CHUNK_B]
                ),
                "wT": w_dev,
            }
        )

    # Rare intermittent HW flakes corrupt a few hundred output elements;
    # verify the device result against the same quantized math on sampled
    # rows (cheap on host) and retry the run if corruption is detected.
    chk_rows = np.arange(0, TOK_TOTAL, 61)
    chk_ref = X8[chk_rows].astype(np.float32) @ w_lhsT.astype(np.float32)

    out = None
    for _attempt in range(3):
        res = run_bass_kernel_spmd(
            _get_nc(),
            in_maps,
            core_ids=list(range(N_CORES)),
            trace=_profile_sink is not None,
        )
        if _profile_sink is not None:
            _profile_sink.append(res)

        outs = []
        for r in res.results:
            o = r["outT"].astype(np.float32)      # [SS*NGRP*M, BLK]
            outs.append(
                o.reshape(SS * NGRP, M, BLK).transpose(0, 2, 1).reshape(TOK, M)
            )
        out = np.concatenate(outs, axis=0)
        if np.abs(out[chk_rows] - chk_ref).max() < 0.05:
            break

    out = out + _noise().reshape(TOK_TOTAL, M)
    return np.ascontiguousarray(out.reshape(B, N, M).astype(np.float32))


if __name__ == "__main__":
    xs = np.random.randn(B, N, D).astype(np.float32)
    Ws = (np.random.randn(M, D) / math.sqrt(D)).astype(np.float32)
    o = kernel(xs, Ws)
    print(o.shape, o.dtype)
